# revision 1
# baseline (speedup 1.0000x reference)
"""Trainium2 Bass kernel for nn_DecoderStack (self-attn + cross-attn +
2-layer GELU FFN, shared decoder LN), 8-core data-parallel.

Sharding: 8 cores = 4 batches x 2 query-halves. Core c handles batch b=c//2,
query half h=c%2 (1024 tokens); K/V context is the full 2048 tokens of its
batch element (inputs only; no collectives).

Math restructuring (exact, up to float32r rounding):
  * softmax is invariant to the K-bias term, so  scores.T = x_kvT @ P  with
    P = (wq @ wk.T).T @ q_in + (wk @ bq)  — a single 1024-token projection
    replaces Q-proj and the 2048-token K-proj (host precomputes wq@wk.T).
  * PV is reassociated:  U = wv.T @ G + bv*denom,  G = x_tok.T-contraction
    of E  — the 2048-token V-proj becomes a 1024-token projection of G.

Layout: activations feature-major [D, S] (D on partitions); all matmuls in
float32r (TF32-like, full PE rate); scores transposed [t, s]; softmax
denominator via ones-column matmuls; LN stats via all-ones [128,128]
stationary matmuls whose sums land replicated on every partition. The
intermediate G stays in SBUF; only X1/X2 round-trip through DRAM.
"""
import sys
for _p in ("/opt/trn_rl_repo", "/root/.axon_site/_ro/trn_rl_repo"):
    if _p not in sys.path:
        sys.path.append(_p)

import numpy as np

import concourse.bass as bass
import concourse.tile as tile
from concourse import bacc, mybir
from concourse.bass_utils import run_bass_kernel_spmd

f32 = mybir.dt.float32
f32r = mybir.dt.float32r
AF = mybir.ActivationFunctionType
ALU = mybir.AluOpType

N_CORES = 8
B, S, T, D = 4, 2048, 2048, 1024
SH = S // 2          # per-core query tokens
KD = D // 128        # 8 d-tiles
TM = T // 128        # 16 t-tiles
SCALE = 1.0 / 8.0
LN_EPS = 1e-5
LN_RD = 1.0 / D

COLS = ["cp1", "cp2", "bv1", "bv2", "gm1", "bm1", "gm2", "bm2",
        "gd", "bd", "fb0", "fb1"]
NCOL = len(COLS)
ONES_COL = NCOL * 8  # last column of the packed cols input


def build_decoder(nc, taps=False, reps=0):
    """Emit the full per-core decoder program. Returns tap tensor names."""
    def din(name, shape, dt=f32r):
        return nc.dram_tensor(name, shape, dt, kind="ExternalInput").ap()

    xq = din("xq", [KD, 128, SH])          # x[b,half].T feature-major
    xkv = din("xkv", [KD, 128, T])         # x[b].T full (feature-major)
    ykv = din("ykv", [KD, 128, T])         # y[b].T full
    xtok = din("xtok", [TM, 128, D])       # x[b] token-major
    ytok = din("ytok", [TM, 128, D])       # y[b] token-major
    w = {n: din("w_" + n, [D, D]) for n in
         ["p1", "v1", "p2", "v2", "f0", "f1"]}
    cols_in = din("cols", [128, NCOL * 8 + 1 + 128], f32r)  # + ones col + ones128
    out = nc.dram_tensor("out", [KD, 128, SH], f32, kind="ExternalOutput").ap()

    tap_names = []

    with tile.TileContext(nc, pool_alloc_mode="queue") as tc:
        import contextlib
        rep_ctx = tc.For_i(0, reps, 1) if reps else contextlib.nullcontext()
        es = []

        def open_pool(name, bufs=1, space="SBUF"):
            cm = tc.tile_pool(name=name, bufs=bufs, space=space)
            pool = cm.__enter__()
            es.append(cm)
            return pool

        rep_ctx.__enter__()
        p_w = open_pool("w", bufs=2)          # weight halves [128,8,512] 16K
        p_st4 = open_pool("st4", bufs=3)      # [128,8,128] tile streams 4K
        p_stage = open_pool("stage", bufs=4)  # [128,1024] staging 4K
        p_bc = open_pool("bc", bufs=2)        # [128,1024] persist stats 4K
        p_rows = open_pool("rows", bufs=1)    # [1,1024] rows 4K
        p_cmn = open_pool("cmn", bufs=1)      # cols + ones128 4K
        p_act = open_pool("act", bufs=1)      # slotA 32K + slotB 64K
        p_dram = open_pool("dram", bufs=1, space="DRAM")
        p_psm = open_pool("psm", bufs=4, space="PSUM")   # [128,512]
        p_psr = open_pool("psr", bufs=4, space="PSUM")   # [128,512]

        cols_sb = p_cmn.tile([128, NCOL * 8 + 1 + 128], f32r, name="cols_sb")
        nc.sync.dma_start(cols_sb[:], cols_in)
        ones_sb = cols_sb[:, ONES_COL:ONES_COL + 1]      # [128,1] ones
        ones128 = cols_sb[:, ONES_COL + 1:ONES_COL + 129]  # [128,128] ones

        def col(name, j):
            c = COLS.index(name)
            return cols_sb[:, c * 8 + j: c * 8 + j + 1].bitcast(f32)

        def tap(name, ap_src, shape, dt_src=f32r):
            if not taps:
                return
            t = nc.dram_tensor("tap_" + name, shape, f32,
                               kind="ExternalOutput").ap()
            tap_names.append("tap_" + name)
            nc.sync.dma_start(t, ap_src.bitcast(f32) if dt_src == f32r else ap_src)

        def load_w_halves(wap):
            """Weight [D, D] as two halves [128, 8, 512] (d_out split)."""
            wr = wap.rearrange("(ko kp) d -> kp ko d", kp=128)
            halves = []
            for hf in range(2):
                t = p_w.tile([128, KD, 512], f32r, tag="w", name=f"wh{hf}")
                nc.sync.dma_start(t[:], wr[:, :, hf * 512:(hf + 1) * 512])
                halves.append(t)
            return halves

        def proj(out_write, wap, rhs_src, n_tok):
            """Feature-major projection: psum[m-tile, 512chunk] = w.T @ rhs.

            out_write(m, tch, ps): epilogue for the [128,512] PSUM tile.
            rhs_src: DRAM AP [KD, 128, n_tok] or SBUF tile [128, KD, n_tok].
            """
            wh = load_w_halves(wap)
            nch = n_tok // 512
            from_dram = rhs_src.space == bass.MemorySpace.DRAM
            pk_cm = tc.tile_pool(name="kvch", bufs=2)
            pk = pk_cm.__enter__()
            for tch in range(nch):
                sl = slice(tch * 512, (tch + 1) * 512)
                if from_dram:
                    kvc = pk.tile([128, KD, 512], f32r, tag="kv", name="kvc")
                    nc.sync.dma_start(
                        kvc[:], rhs_src[:, :, sl].rearrange("ko p s -> p ko s"))
                    rhs = lambda k: kvc[:, k, :]
                else:
                    rhs = lambda k, sl=sl: rhs_src[:, k, sl]
                for m in range(KD):
                    ps = p_psm.tile([128, 512], f32, tag="mm", name="proj_ps")
                    whf = wh[m // 4]
                    ml = m % 4
                    for k in range(KD):
                        nc.tensor.matmul(
                            ps[:], lhsT=whf[:, k, ml * 128:(ml + 1) * 128],
                            rhs=rhs(k), start=(k == 0), stop=(k == KD - 1))
                    out_write(m, tch, ps)
            pk_cm.__exit__(None, None, None)

        def layernorm(z_sb, g_cb, b_cb, out_cb):
            """LN over the feature dim (128 partitions x KD) of [128,KD,SH].

            Stats matmuls use an all-ones [128,128] stationary so sums land
            replicated on every partition (no broadcast step needed).
            """
            ps_s = [p_psr.tile([128, 512], f32, tag="row", name=f"lns{i}")
                    for i in range(2)]
            ps_q = [p_psr.tile([128, 512], f32, tag="row", name=f"lnq{i}")
                    for i in range(2)]
            for m in range(KD):
                sq = p_stage.tile([128, 1024], f32r, tag="stage", name="lnsq")
                nc.scalar.activation(sq[:], z_sb[:, m, :], AF.Square)
                for sch in range(2):
                    sl = slice(sch * 512, (sch + 1) * 512)
                    nc.tensor.matmul(ps_s[sch][:], lhsT=ones128[:],
                                     rhs=z_sb[:, m, sl],
                                     start=(m == 0), stop=(m == KD - 1))
                    nc.tensor.matmul(ps_q[sch][:], lhsT=ones128[:],
                                     rhs=sq[:, sl],
                                     start=(m == 0), stop=(m == KD - 1))
            mean = p_stage.tile([128, 1024], f32, tag="stage", name="mean")
            vp = p_stage.tile([128, 1024], f32, tag="stage", name="vp")
            for sch in range(2):
                sl = slice(sch * 512, (sch + 1) * 512)
                nc.vector.tensor_scalar(mean[:, sl], ps_s[sch][:], LN_RD, None,
                                        op0=ALU.mult)
                nc.vector.tensor_scalar(vp[:, sl], ps_q[sch][:], LN_RD, LN_EPS,
                                        op0=ALU.mult, op1=ALU.add)
            msq = p_stage.tile([128, 1024], f32, tag="stage", name="msq")
            nc.vector.tensor_mul(msq[:], mean[:], mean[:])
            varc = p_stage.tile([128, 1024], f32, tag="stage", name="varc")
            nc.vector.tensor_sub(varc[:], vp[:], msq[:])
            std = p_stage.tile([128, 1024], f32, tag="stage", name="std")
            nc.scalar.activation(std[:], varc[:], AF.Sqrt)
            rstd = p_bc.tile([128, 1024], f32, tag="bc", name="rstd")
            nc.vector.reciprocal(rstd[:], std[:])
            cr = p_bc.tile([128, 1024], f32, tag="bc", name="cr")
            nc.vector.tensor_mul(cr[:], mean[:], rstd[:])
            for m in range(KD):
                t1 = p_stage.tile([128, 1024], f32, tag="stage", name="ln_t1")
                nc.vector.tensor_mul(t1[:], z_sb[:, m, :].bitcast(f32), rstd[:])
                t2 = p_stage.tile([128, 1024], f32, tag="stage", name="ln_t2")
                nc.vector.tensor_sub(t2[:], t1[:], cr[:])
                nc.vector.tensor_scalar(out_cb(m), t2[:], g_cb(m), b_cb(m),
                                        op0=ALU.mult, op1=ALU.add)

        def attention_block(qin_d, kvF_d, kvT_d, wP, wV, cpn, bvn,
                            gmn, bmn, xout_d, blk):
            # ---- P projection (slot A): P = wP.T @ qin + cp ----
            p_sb = p_act.tile([128, KD, SH], f32r, tag="slotA", name="p_sb")

            def pwrite(m, sch, ps):
                nc.vector.tensor_scalar(p_sb[:, m, sch * 512:(sch + 1) * 512],
                                        ps[:], col(cpn, m), None, op0=ALU.add)
            proj(pwrite, wP, qin_d, SH)
            tap(f"P{blk}", p_sb[:], [128, KD, SH])

            # ---- scores.T = kvF.T-contraction of P ; exp ; denominator ----
            e_sb = p_act.tile([128, TM, SH], f32r, tag="slotB", name="e_sb")
            for tm in range(TM):
                kt = p_st4.tile([128, KD, 128], f32r, tag="st4", name="kt")
                nc.sync.dma_start(
                    kt[:], kvF_d[:, :, tm * 128:(tm + 1) * 128]
                    .rearrange("ko p t -> p ko t"))
                for sch in range(2):
                    sl = slice(sch * 512, (sch + 1) * 512)
                    ps = p_psm.tile([128, 512], f32, tag="mm", name="sc_ps")
                    for k in range(KD):
                        nc.tensor.matmul(ps[:], lhsT=kt[:, k, :],
                                         rhs=p_sb[:, k, sl],
                                         start=(k == 0), stop=(k == KD - 1))
                    nc.scalar.activation(e_sb[:, tm, sl], ps[:], AF.Exp,
                                         scale=SCALE)

            # ---- G = kvT.T-contraction of E (slot A); denom rides along ----
            g_sb = p_act.tile([128, KD, SH], f32r, tag="slotA", name="g_sb")
            ps_d = [p_psr.tile([1, 512], f32, tag="row", name=f"dn{i}")
                    for i in range(2)]
            for tm in range(TM):
                for sch in range(2):
                    sl = slice(sch * 512, (sch + 1) * 512)
                    nc.tensor.matmul(ps_d[sch][:], lhsT=ones_sb,
                                     rhs=e_sb[:, tm, sl],
                                     start=(tm == 0), stop=(tm == TM - 1))
            rden_row = p_rows.tile([1, 1024], f32, tag="row", name="rden_row")
            for sch in range(2):
                sl = slice(sch * 512, (sch + 1) * 512)
                nc.vector.reciprocal(rden_row[:, sl], ps_d[sch][:])
            if taps:
                den_r = p_rows.tile([1, 1024], f32, tag="row", name="den_r")
                for sch in range(2):
                    nc.scalar.copy(den_r[:, sch * 512:(sch + 1) * 512],
                                   ps_d[sch][:])
                tap(f"den{blk}", den_r[:], [1, 1024], f32)
            rden_bc = p_bc.tile([128, 1024], f32, tag="bc", name="rden_bc")
            nc.gpsimd.partition_broadcast(rden_bc[:], rden_row[:])
            for m in range(KD):
                vh = []
                for hfm in range(2):
                    vt = p_st4.tile([128, 8, 128], f32r, tag="st4", name="vh")
                    nc.sync.dma_start(
                        vt[:], kvT_d[hfm * 8:(hfm + 1) * 8, :,
                                     m * 128:(m + 1) * 128]
                        .rearrange("tm p d -> p tm d"))
                    vh.append(vt)
                psu = [p_psm.tile([128, 512], f32, tag="mm", name=f"pv{i}")
                       for i in range(2)]
                for tm in range(TM):
                    vt = vh[tm // 8][:, tm % 8, :]
                    for sch in range(2):
                        sl = slice(sch * 512, (sch + 1) * 512)
                        nc.tensor.matmul(psu[sch][:], lhsT=vt,
                                         rhs=e_sb[:, tm, sl],
                                         start=(tm == 0), stop=(tm == TM - 1))
                for sch in range(2):
                    nc.scalar.copy(g_sb[:, m, sch * 512:(sch + 1) * 512],
                                   psu[sch][:])

            # ---- U = wV.T @ G ; normalize ; +bv ; +resid -> Z (slot B) ----
            z_sb = p_act.tile([128, KD, SH], f32r, tag="slotB", name="z_sb")

            def uwrite(m, sch, ps):
                sl = slice(sch * 512, (sch + 1) * 512)
                rt = p_stage.tile([128, 1024], f32r, tag="stage", name="res_t")
                nc.sync.dma_start(rt[:, 0:512], qin_d[m, :, sl])
                t1 = p_stage.tile([128, 1024], f32, tag="stage", name="pv_t1")
                nc.vector.tensor_mul(t1[:, 0:512], ps[:], rden_bc[:, sl])
                t2 = p_stage.tile([128, 1024], f32, tag="stage", name="pv_t2")
                nc.vector.tensor_add(t2[:, 0:512], t1[:, 0:512],
                                     rt[:, 0:512].bitcast(f32))
                nc.vector.tensor_scalar(z_sb[:, m, sl], t2[:, 0:512],
                                        col(bvn, m), None, op0=ALU.add)
            proj(uwrite, wV, g_sb, SH)
            tap(f"Z1_{blk}", z_sb[:], [128, KD, SH])

            # ---- LN_m (in-place) ; + resid (in-place) ; LN_d -> xout ----
            layernorm(z_sb, lambda m: col(gmn, m), lambda m: col(bmn, m),
                      lambda m: z_sb[:, m, :])
            for m in range(KD):
                for sch in range(2):
                    sl = slice(sch * 512, (sch + 1) * 512)
                    rt = p_stage.tile([128, 1024], f32r, tag="stage",
                                      name="res2_t")
                    nc.sync.dma_start(rt[:, 0:512], qin_d[m, :, sl])
                    nc.vector.tensor_add(z_sb[:, m, sl], z_sb[:, m, sl],
                                         rt[:, 0:512])
            sts = {}

            def xcb(m):
                st = p_stage.tile([128, 1024], f32r, tag="stage", name="xo_st")
                sts[m] = st
                return st[:, 0:SH]
            layernorm(z_sb, lambda m: col("gd", m), lambda m: col("bd", m), xcb)
            for m in range(KD):
                nc.sync.dma_start(xout_d[m, :, :], sts[m][:, 0:SH])

        # ================= decoder =================
        x1_d = p_dram.tile([KD, 128, SH], f32r, tag="x1", name="x1_d")
        attention_block(xq, xkv, xtok, w["p1"], w["v1"], "cp1", "bv1",
                        "gm1", "bm1", x1_d, 1)
        x2_d = p_dram.tile([KD, 128, SH], f32r, tag="x2", name="x2_d")
        attention_block(x1_d, ykv, ytok, w["p2"], w["v2"], "cp2", "bv2",
                        "gm2", "bm2", x2_d, 2)

        # ================= FFN =================
        h1 = p_act.tile([128, KD, SH], f32r, tag="slotA", name="h1")

        def h1w(m, sch, ps):
            nc.scalar.activation(h1[:, m, sch * 512:(sch + 1) * 512], ps[:],
                                 AF.Gelu, bias=col("fb0", m))
        proj(h1w, w["f0"], x2_d, SH)

        z5 = p_act.tile([128, KD, SH], f32r, tag="slotB", name="z5")

        def h2w(m, sch, ps):
            sl = slice(sch * 512, (sch + 1) * 512)
            t1 = p_stage.tile([128, 1024], f32, tag="stage", name="h2_t")
            nc.scalar.activation(t1[:, 0:512], ps[:], AF.Gelu,
                                 bias=col("fb1", m))
            rt = p_stage.tile([128, 1024], f32r, tag="stage", name="resf_t")
            nc.sync.dma_start(rt[:, 0:512], x2_d[m, :, sl])
            nc.vector.tensor_add(z5[:, m, sl], t1[:, 0:512],
                                 rt[:, 0:512].bitcast(f32))
        proj(h2w, w["f1"], h1, SH)

        outs = {}

        def out_cb(m):
            st = p_stage.tile([128, 1024], f32r, tag="stage", name="out_st")
            outs[m] = st
            return st[:, 0:SH]
        layernorm(z5, lambda m: col("gd", m), lambda m: col("bd", m), out_cb)
        for m in range(KD):
            nc.sync.dma_start(out[m, :, :], outs[m][:, 0:SH].bitcast(f32))

        for cm in reversed(es):
            cm.__exit__(None, None, None)
        rep_ctx.__exit__(None, None, None)

    nc.compile()
    return tap_names


def _prep_inputs(inputs):
    """Host-side sharding + weight folding: returns in_maps (8 dicts)."""
    f64 = lambda k: np.asarray(inputs[k], np.float64)
    x, y = inputs["x"], inputs["y"]
    # folded attention weights: P = (wq@wk.T).T @ qin + wk@bq
    wp1 = (f64("wq_m") @ f64("wk_m").T).astype(np.float32)
    cp1 = (f64("wk_m") @ f64("bq_m")).astype(np.float32)
    wp2 = (f64("wq_c") @ f64("wk_c").T).astype(np.float32)
    cp2 = (f64("wk_c") @ f64("bq_c")).astype(np.float32)
    colvecs = {
        "cp1": cp1, "cp2": cp2,
        "bv1": inputs["bv_m"], "bv2": inputs["bv_c"],
        "gm1": inputs["g_m"], "bm1": inputs["b_m"],
        "gm2": inputs["g_c"], "bm2": inputs["b_c"],
        "gd": inputs["g_d"], "bd": inputs["b_d"],
        "fb0": inputs["f0_b"], "fb1": inputs["f1_b"],
    }
    cols = np.empty((128, NCOL * 8 + 1 + 128), np.float32)
    for c, n in enumerate(COLS):
        cols[:, c * 8:(c + 1) * 8] = np.asarray(colvecs[n], np.float32) \
            .reshape(KD, 128).T
    cols[:, ONES_COL:] = 1.0
    shared = {
        "w_p1": wp1, "w_p2": wp2,
        "w_v1": np.asarray(inputs["wv_m"], np.float32),
        "w_v2": np.asarray(inputs["wv_c"], np.float32),
        "w_f0": np.asarray(inputs["f0_w"], np.float32),
        "w_f1": np.asarray(inputs["f1_w"], np.float32),
        "cols": cols,
    }
    in_maps = []
    for c in range(N_CORES):
        b, h = c // 2, c % 2
        xb = np.asarray(x[b], np.float32)
        yb = np.asarray(y[b], np.float32)
        xT = np.ascontiguousarray(xb.T)  # [D, T]
        yT = np.ascontiguousarray(yb.T)
        m = dict(shared)
        m["xkv"] = xT.reshape(KD, 128, T)
        m["ykv"] = yT.reshape(KD, 128, T)
        m["xtok"] = np.ascontiguousarray(xb).reshape(TM, 128, D)
        m["ytok"] = np.ascontiguousarray(yb).reshape(TM, 128, D)
        m["xq"] = np.ascontiguousarray(
            xT[:, h * SH:(h + 1) * SH]).reshape(KD, 128, SH)
        in_maps.append(m)
    return in_maps


def kernel(**inputs):
    nc = bacc.Bacc("TRN2", target_bir_lowering=False, debug=False,
                   num_devices=N_CORES)
    build_decoder(nc, taps=False)
    in_maps = _prep_inputs(inputs)
    res = run_bass_kernel_spmd(nc, in_maps, core_ids=list(range(N_CORES)),
                               trace=False)
    out = np.empty((B, S, D), np.float32)
    for c in range(N_CORES):
        b, h = c // 2, c % 2
        o = res.results[c]["out"].reshape(D, SH)  # feature-major [d, s]
        out[b, h * SH:(h + 1) * SH, :] = o.T
    return out



# revision 4
# speedup vs baseline: 70.9900x; 70.9900x over previous
"""Trainium2 Bass kernel for nn_DecoderStack (self-attn + cross-attn +
2-layer GELU FFN, shared decoder LN), 8-core data-parallel.

Sharding: 8 cores = 4 batches x 2 query-halves. Core c handles batch b=c//2,
query half h=c%2 (1024 tokens); K/V context is the full 2048 tokens of its
batch element (inputs only; no collectives).

Math restructuring (exact, up to rounding):
  * softmax is invariant to the K-bias term, so  scores.T = x_kvT @ P  with
    P = (wq @ wk.T).T @ q_in + (wk @ bq)  — a single 1024-token projection
    replaces Q-proj and the 2048-token K-proj (host precomputes wq@wk.T).
  * PV is reassociated:  U = wv.T @ G + bv*denom,  G = x_tok.T-contraction
    of E.
  * The shared decoder LN at each block boundary is FOLDED into the next
    projection:  W.T @ LN(z) = rstd*(Wg.T @ z - mean*colsum(Wg)) + W.T@b
    with Wg = diag(g)W folded on host; the -mean*colsum term is one extra
    matmul per accumulation group (lhsT with colsums on partition 0 only),
    and rstd scales the PSUM epilogue.  The projection therefore starts as
    soon as z exists — the LN tail runs in parallel on Vector/Scalar.

Layout: activations feature-major [D, S] (D on partitions); residual/LN
stream in f32; all attention/FFN matmul operands in bf16 (fp32 PSUM
accumulate); scores transposed [t, s]; softmax denominator via ones-column
matmuls; LN stats via all-ones [128,128] stationary matmuls whose sums land
replicated on every partition; reciprocals via Exp(-Log) on ScalarE.
Everything stays in SBUF between stages (no DRAM round-trips).
"""
import sys
for _p in ("/opt/trn_rl_repo", "/root/.axon_site/_ro/trn_rl_repo"):
    if _p not in sys.path:
        sys.path.append(_p)

import numpy as np
import ml_dtypes

import concourse.bass as bass
import concourse.tile as tile
from concourse import bacc, mybir
from concourse.bass_utils import run_bass_kernel_spmd

f32 = mybir.dt.float32
f32r = mybir.dt.float32r
bf16 = mybir.dt.bfloat16
AF = mybir.ActivationFunctionType
ALU = mybir.AluOpType

N_CORES = 8
B, S, T, D = 4, 2048, 2048, 1024
SH = S // 2          # per-core query tokens
KD = D // 128        # 8 d-tiles
TM = T // 128        # 16 t-tiles
SCALE = 1.0 / 8.0
LN_EPS = 1e-5
LN_RD = 1.0 / D

COLS = ["cp1", "cp2", "bv1", "bv2", "gm1", "bm1", "gm2", "bm2",
        "gd", "bd", "fb0", "fb1"]
NCOL = len(COLS)
ONES128 = NCOL * 8  # ones [128,128] block at the end of cols


def build_decoder(nc, taps=False, reps=0):
    """Emit the full per-core decoder program. Returns tap tensor names."""
    def din(name, shape, dt=bf16):
        return nc.dram_tensor(name, shape, dt, kind="ExternalInput").ap()

    xqf = din("xqf", [KD, 128, SH], f32r)   # x[b,half].T f32 (residual src)
    xqb = din("xqb", [KD, 128, SH])         # same, bf16 (proj rhs)
    xkv = din("xkv", [KD, 128, T])          # x[b].T bf16 feature-major
    ykv = din("ykv", [KD, 128, T])
    xtok = din("xtok", [TM, 128, D])        # x[b] bf16 token-major
    ytok = din("ytok", [TM, 128, D])
    w = {n: din("w_" + n, [D, D]) for n in
         ["p1", "v1", "p2", "v2", "f0", "f1"]}
    uxw = din("uxw", [128, 2 * D])          # -colsum lhsT rows (p2, f0)
    cols_in = din("cols", [128, NCOL * 8 + 128], f32r)
    colsb_in = din("colsb", [128, 16])      # bf16 consts: col0 = ones
    out = nc.dram_tensor("out", [KD, 128, SH], f32, kind="ExternalOutput").ap()

    tap_names = []

    with tile.TileContext(nc, pool_alloc_mode="queue") as tc:
        import contextlib
        rep_ctx = tc.For_i(0, reps, 1) if reps else contextlib.nullcontext()
        es = []

        def open_pool(name, bufs=1, space="SBUF"):
            cm = tc.tile_pool(name=name, bufs=bufs, space=space)
            pool = cm.__enter__()
            es.append(cm)
            return pool

        rep_ctx.__enter__()
        p_w = open_pool("w", bufs=4)          # weight halves [128,8,512]b 8K
        p_st4 = open_pool("st4", bufs=4)      # [128,8,128]b tile streams 2K
        p_stage = open_pool("stage", bufs=4)  # [128,1024] staging 4K
        p_bc = open_pool("bc", bufs=6)        # [128,1024] persist stats 4K
        p_rows = open_pool("rows", bufs=2)    # [1,1024] rows 4K
        p_cmn = open_pool("cmn", bufs=1)      # cols + colsb + uxw
        p_act = open_pool("act", bufs=1)      # slotA 16K + slotB 32K
        p_zbf = open_pool("zbf", bufs=1)      # bf16 proj rhs 16K
        p_x = open_pool("x", bufs=1)          # f32 resid 32K
        p_psm = open_pool("psm", bufs=4, space="PSUM")   # [128,512]
        p_psr = open_pool("psr", bufs=4, space="PSUM")   # [128,512]

        cols_sb = p_cmn.tile([128, NCOL * 8 + 128], f32r, name="cols_sb")
        nc.sync.dma_start(cols_sb[:], cols_in)
        colsb_sb = p_cmn.tile([128, 16], bf16, name="colsb_sb")
        nc.sync.dma_start(colsb_sb[:], colsb_in)
        ux_sb = p_cmn.tile([128, 2 * D], bf16, name="ux_sb")
        nc.sync.dma_start(ux_sb[:], uxw)
        ones128 = cols_sb[:, ONES128:ONES128 + 128]  # [128,128] f32r ones
        onesb = colsb_sb[:, 0:1]                     # [128,1] bf16 ones

        def col(name, j):
            c = COLS.index(name)
            return cols_sb[:, c * 8 + j: c * 8 + j + 1].bitcast(f32)

        def tap(name, ap_src, shape, dt=f32):
            if not taps:
                return
            t = nc.dram_tensor("tap_" + name, shape, dt,
                               kind="ExternalOutput").ap()
            tap_names.append("tap_" + name)
            nc.sync.dma_start(t, ap_src)

        def load_w_halves(wap):
            """Weight [D, D] bf16 as two halves [128, 8, 512] (d_out split)."""
            wr = wap.rearrange("(ko kp) d -> kp ko d", kp=128)
            halves = []
            for hf in range(2):
                t = p_w.tile([128, KD, 512], bf16, tag="w", name=f"wh{hf}")
                nc.sync.dma_start(t[:], wr[:, :, hf * 512:(hf + 1) * 512])
                halves.append(t)
            return halves

        def proj(out_write, wap, rhs_sb, fold=None):
            """Feature-major projection: psum[m-tile, 512chunk] = w.T @ rhs.

            rhs_sb: SBUF bf16 tile [128, KD, SH].
            fold: (stats, ux_base) applies the LN-fold correction matmul
            (-colsum(Wg)*mean) as a 9th accumulation-group matmul.
            """
            wh = load_w_halves(wap)
            for tch in range(2):
                sl = slice(tch * 512, (tch + 1) * 512)
                for m in range(KD):
                    ps = p_psm.tile([128, 512], f32, tag="mm", name="proj_ps")
                    whf = wh[m // 4]
                    ml = m % 4
                    for k in range(KD):
                        nc.tensor.matmul(
                            ps[:], lhsT=whf[:, k, ml * 128:(ml + 1) * 128],
                            rhs=rhs_sb[:, k, sl], start=(k == 0),
                            stop=(k == KD - 1 and fold is None))
                    if fold is not None:
                        st, ux_base = fold
                        nc.tensor.matmul(
                            ps[:],
                            lhsT=ux_sb[:, ux_base + m * 128:
                                       ux_base + (m + 1) * 128],
                            rhs=st["mean_bf"][:, sl],
                            start=False, stop=True)
                    out_write(m, tch, ps)

        def ln_stats(z_sb, want_bf=False):
            """LN stats over the feature dim of [128,KD,SH] (f32r bits).

            Stats matmuls use an all-ones [128,128] stationary so sums land
            replicated on every partition. Returns dict with rstd (f32
            [128,1024] bc), cr = mean*rstd, and optionally mean_bf (bf16)."""
            ps_s = [p_psr.tile([128, 512], f32, tag="row", name=f"lns{i}")
                    for i in range(2)]
            ps_q = [p_psr.tile([128, 512], f32, tag="row", name=f"lnq{i}")
                    for i in range(2)]
            for m in range(KD):
                sq = p_stage.tile([128, 1024], f32r, tag="stage", name="lnsq")
                nc.scalar.activation(sq[:], z_sb[:, m, :], AF.Square)
                for sch in range(2):
                    sl = slice(sch * 512, (sch + 1) * 512)
                    nc.tensor.matmul(ps_s[sch][:], lhsT=ones128[:],
                                     rhs=z_sb[:, m, sl],
                                     start=(m == 0), stop=(m == KD - 1))
                    nc.tensor.matmul(ps_q[sch][:], lhsT=ones128[:],
                                     rhs=sq[:, sl],
                                     start=(m == 0), stop=(m == KD - 1))
            st = {}
            mean = p_stage.tile([128, 1024], f32, tag="stage", name="mean")
            vp = p_stage.tile([128, 1024], f32, tag="stage", name="vp")
            for sch in range(2):
                sl = slice(sch * 512, (sch + 1) * 512)
                nc.vector.tensor_scalar(mean[:, sl], ps_s[sch][:], LN_RD, None,
                                        op0=ALU.mult)
                nc.vector.tensor_scalar(vp[:, sl], ps_q[sch][:], LN_RD, LN_EPS,
                                        op0=ALU.mult, op1=ALU.add)
            if want_bf:
                mean_bf = p_bc.tile([128, 1024], bf16, tag="bcb",
                                    name="mean_bf")
                nc.vector.tensor_copy(mean_bf[:], mean[:])
                st["mean_bf"] = mean_bf
            msq = p_stage.tile([128, 1024], f32, tag="stage", name="msq")
            nc.vector.tensor_mul(msq[:], mean[:], mean[:])
            varc = p_stage.tile([128, 1024], f32, tag="stage", name="varc")
            nc.vector.tensor_sub(varc[:], vp[:], msq[:])
            lgv = p_stage.tile([128, 1024], f32, tag="stage", name="lgv")
            nc.scalar.activation(lgv[:], varc[:], AF.Ln)
            rstd = p_bc.tile([128, 1024], f32, tag="bc", name="rstd")
            nc.scalar.activation(rstd[:], lgv[:], AF.Exp, scale=-0.5)
            cr = p_bc.tile([128, 1024], f32, tag="bc", name="cr")
            nc.vector.tensor_mul(cr[:], mean[:], rstd[:])
            st["rstd"] = rstd
            st["cr"] = cr
            return st

        def ln_apply(z_sb, st, g_cb, b_cb, out_cb):
            """out[m] = (z[m]*rstd - cr)*g + b, per 128-feature chunk m."""
            for m in range(KD):
                t1 = p_stage.tile([128, 1024], f32, tag="stage", name="ln_t1")
                nc.vector.tensor_mul(t1[:], z_sb[:, m, :].bitcast(f32),
                                     st["rstd"][:])
                t2 = p_stage.tile([128, 1024], f32, tag="stage", name="ln_t2")
                nc.vector.tensor_sub(t2[:], t1[:], st["cr"][:])
                nc.vector.tensor_scalar(out_cb(m), t2[:], g_cb(m), b_cb(m),
                                        op0=ALU.mult, op1=ALU.add)

        def attention_core(qres_sb, rhs_bf, kvF_d, kvT_d, wP, wV, cpn, bvn,
                           gmn, bmn, blk, fold=None):
            """One attention block; returns z4 (f32r bits, [128,KD,SH]):
            z4 = LN_gm,bm(U' + qres) + qres, where U' is attention out."""
            # ---- P projection (slot A): P = [rstd*] wP.T@rhs + col ----
            p_sb = p_act.tile([128, KD, SH], bf16, tag="slotA", name="p_sb")

            def pwrite(m, tch, ps):
                sl = slice(tch * 512, (tch + 1) * 512)
                if fold is not None:
                    st = fold[0]
                    t1 = p_stage.tile([128, 1024], f32, tag="stage",
                                      name="pw_t1")
                    nc.vector.tensor_mul(t1[:, 0:512], ps[:],
                                         st["rstd"][:, sl])
                    nc.vector.tensor_scalar(p_sb[:, m, sl], t1[:, 0:512],
                                            col(cpn, m), None, op0=ALU.add)
                else:
                    nc.vector.tensor_scalar(p_sb[:, m, sl], ps[:],
                                            col(cpn, m), None, op0=ALU.add)
            proj(pwrite, wP, rhs_bf, fold=fold)
            tap(f"P{blk}", p_sb[:], [128, KD, SH], bf16)

            # ---- scores.T = kvF.T-contraction of P ; exp -> E (slot B) ----
            e_sb = p_act.tile([128, TM, SH], bf16, tag="slotB", name="e_sb")
            for tm in range(TM):
                kt = p_st4.tile([128, KD, 128], bf16, tag="st4", name="kt")
                nc.sync.dma_start(
                    kt[:], kvF_d[:, :, tm * 128:(tm + 1) * 128]
                    .rearrange("ko p t -> p ko t"))
                for sch in range(2):
                    sl = slice(sch * 512, (sch + 1) * 512)
                    ps = p_psm.tile([128, 512], f32, tag="mm", name="sc_ps")
                    for k in range(KD):
                        nc.tensor.matmul(ps[:], lhsT=kt[:, k, :],
                                         rhs=p_sb[:, k, sl],
                                         start=(k == 0), stop=(k == KD - 1))
                    nc.scalar.activation(e_sb[:, tm, sl], ps[:], AF.Exp,
                                         scale=SCALE)

            # ---- denominator: ones.T-contraction of E; rden = 1/den ----
            ps_d = [p_psr.tile([1, 512], f32, tag="row", name=f"dn{i}")
                    for i in range(2)]
            for tm in range(TM):
                for sch in range(2):
                    sl = slice(sch * 512, (sch + 1) * 512)
                    nc.tensor.matmul(ps_d[sch][:], lhsT=onesb,
                                     rhs=e_sb[:, tm, sl],
                                     start=(tm == 0), stop=(tm == TM - 1))
            rden_row = p_rows.tile([1, 1024], f32, tag="row", name="rden_row")
            lden_row = p_rows.tile([1, 1024], f32, tag="row", name="lden_row")
            for sch in range(2):
                sl = slice(sch * 512, (sch + 1) * 512)
                nc.scalar.activation(lden_row[:, sl], ps_d[sch][:], AF.Ln)
                nc.scalar.activation(rden_row[:, sl], lden_row[:, sl],
                                     AF.Exp, scale=-1.0)
            if taps:
                den_r = p_stage.tile([1, 1024], f32, tag="stage", name="den_r")
                for sch in range(2):
                    nc.scalar.copy(den_r[:, sch * 512:(sch + 1) * 512],
                                   ps_d[sch][:])
                tap(f"den{blk}", den_r[:], [1, 1024], f32)
            rden_bc = p_bc.tile([128, 1024], f32, tag="bc", name="rden_bc")
            nc.gpsimd.partition_broadcast(rden_bc[:], rden_row[:])

            # ---- G = kvT.T-contraction of E (slot A) ----
            g_sb = p_act.tile([128, KD, SH], bf16, tag="slotA", name="g_sb")
            for m in range(KD):
                vh = []
                for hfm in range(2):
                    vt = p_st4.tile([128, 8, 128], bf16, tag="st4", name="vh")
                    nc.sync.dma_start(
                        vt[:], kvT_d[hfm * 8:(hfm + 1) * 8, :,
                                     m * 128:(m + 1) * 128]
                        .rearrange("tm p d -> p tm d"))
                    vh.append(vt)
                psu = [p_psm.tile([128, 512], f32, tag="mm", name=f"pv{i}")
                       for i in range(2)]
                for tm in range(TM):
                    vt = vh[tm // 8][:, tm % 8, :]
                    for sch in range(2):
                        sl = slice(sch * 512, (sch + 1) * 512)
                        nc.tensor.matmul(psu[sch][:], lhsT=vt,
                                         rhs=e_sb[:, tm, sl],
                                         start=(tm == 0), stop=(tm == TM - 1))
                for sch in range(2):
                    nc.scalar.copy(g_sb[:, m, sch * 512:(sch + 1) * 512],
                                   psu[sch][:])

            # ---- U = wV.T @ G ; *rden ; +bv ; +resid -> Z (slot B) ----
            z_sb = p_act.tile([128, KD, SH], f32r, tag="slotB", name="z_sb")

            def uwrite(m, tch, ps):
                sl = slice(tch * 512, (tch + 1) * 512)
                t1 = p_stage.tile([128, 1024], f32, tag="stage", name="pv_t1")
                nc.vector.tensor_mul(t1[:, 0:512], ps[:], rden_bc[:, sl])
                t2 = p_stage.tile([128, 1024], f32, tag="stage", name="pv_t2")
                nc.vector.tensor_scalar(t2[:, 0:512], t1[:, 0:512],
                                        col(bvn, m), None, op0=ALU.add)
                nc.vector.tensor_add(z_sb[:, m, sl], t2[:, 0:512],
                                     qres_sb[:, m, sl])
            proj(uwrite, wV, g_sb)
            tap(f"Z1_{blk}", z_sb[:].bitcast(f32), [128, KD, SH])

            # ---- LN_m (in-place) ; + resid (in-place) -> z4 ----
            stm = ln_stats(z_sb)
            ln_apply(z_sb, stm, lambda m: col(gmn, m), lambda m: col(bmn, m),
                     lambda m: z_sb[:, m, :])
            for m in range(KD):
                nc.vector.tensor_add(z_sb[:, m, :], z_sb[:, m, :],
                                     qres_sb[:, m, :])
            return z_sb

        def boundary(z4, name):
            """LN_d fold prep at a block boundary: stats of z4, bf16 copy of
            z4 for the next projection, explicit x = LN_d(z4) for residuals."""
            st = ln_stats(z4, want_bf=True)
            z4_bf = p_zbf.tile([128, KD, SH], bf16, tag="zbf",
                               name=f"zbf_{name}")
            for m in range(KD):
                nc.vector.tensor_copy(z4_bf[:, m, :], z4[:, m, :].bitcast(f32))
            x_new = p_x.tile([128, KD, SH], f32r, tag="x", name=f"x_{name}")
            ln_apply(z4, st, lambda m: col("gd", m), lambda m: col("bd", m),
                     lambda m: x_new[:, m, :])
            return st, z4_bf, x_new

        # ================= decoder =================
        x_sb = p_x.tile([128, KD, SH], f32r, tag="x", name="x_xq")
        for tch in range(2):
            sl = slice(tch * 512, (tch + 1) * 512)
            nc.sync.dma_start(x_sb[:, :, sl],
                              xqf[:, :, sl].rearrange("ko p s -> p ko s"))
        xq_bf = p_zbf.tile([128, KD, SH], bf16, tag="zbf", name="xq_bf")
        for tch in range(2):
            sl = slice(tch * 512, (tch + 1) * 512)
            nc.sync.dma_start(xq_bf[:, :, sl],
                              xqb[:, :, sl].rearrange("ko p s -> p ko s"))

        z4a = attention_core(x_sb, xq_bf, xkv, xtok, w["p1"], w["v1"],
                             "cp1", "bv1", "gm1", "bm1", 1)
        st1, z4a_bf, x1 = boundary(z4a, "b1")
        z4b = attention_core(x1, z4a_bf, ykv, ytok, w["p2"], w["v2"],
                             "cp2", "bv2", "gm2", "bm2", 2, fold=(st1, 0))
        st2, z4b_bf, x2 = boundary(z4b, "b2")

        # ================= FFN =================
        h1 = p_act.tile([128, KD, SH], bf16, tag="slotA", name="h1")

        def h1w(m, tch, ps):
            sl = slice(tch * 512, (tch + 1) * 512)
            t1 = p_stage.tile([128, 1024], f32, tag="stage", name="h1_t1")
            nc.vector.tensor_mul(t1[:, 0:512], ps[:], st2["rstd"][:, sl])
            nc.scalar.activation(h1[:, m, sl], t1[:, 0:512], AF.Gelu,
                                 bias=col("fb0", m))
        proj(h1w, w["f0"], z4b_bf, fold=(st2, D))

        z5 = p_act.tile([128, KD, SH], f32r, tag="slotB", name="z5")

        def h2w(m, tch, ps):
            sl = slice(tch * 512, (tch + 1) * 512)
            t1 = p_stage.tile([128, 1024], f32, tag="stage", name="h2_t")
            nc.scalar.activation(t1[:, 0:512], ps[:], AF.Gelu,
                                 bias=col("fb1", m))
            nc.vector.tensor_add(z5[:, m, sl], t1[:, 0:512], x2[:, m, sl])
        proj(h2w, w["f1"], h1)

        st3 = ln_stats(z5)
        for m in range(KD):
            stt = p_stage.tile([128, 1024], f32r, tag="stage", name="out_st")
            t1 = p_stage.tile([128, 1024], f32, tag="stage", name="fo_t1")
            nc.vector.tensor_mul(t1[:], z5[:, m, :].bitcast(f32), st3["rstd"][:])
            t2 = p_stage.tile([128, 1024], f32, tag="stage", name="fo_t2")
            nc.vector.tensor_sub(t2[:], t1[:], st3["cr"][:])
            nc.vector.tensor_scalar(stt[:, 0:SH], t2[:], col("gd", m),
                                    col("bd", m), op0=ALU.mult, op1=ALU.add)
            nc.sync.dma_start(out[m, :, :], stt[:, 0:SH].bitcast(f32))

        for cm in reversed(es):
            cm.__exit__(None, None, None)
        rep_ctx.__exit__(None, None, None)

    nc.compile()
    return tap_names


def _prep_inputs(inputs):
    """Host-side sharding + weight folding: returns in_maps (8 dicts)."""
    f64 = lambda k: np.asarray(inputs[k], np.float64)
    bf = lambda a: np.asarray(a, dtype=ml_dtypes.bfloat16)
    x, y = inputs["x"], inputs["y"]
    gd, bd = f64("g_d"), f64("b_d")
    # folded attention weights: P = (wq@wk.T).T @ qin + wk@bq
    wp1 = f64("wq_m") @ f64("wk_m").T
    cp1 = f64("wk_m") @ f64("bq_m")
    wp2 = f64("wq_c") @ f64("wk_c").T
    wp2g = gd[:, None] * wp2
    cp2 = f64("wk_c") @ f64("bq_c") + wp2.T @ bd
    f0 = f64("f0_w")
    f0g = gd[:, None] * f0
    fb0 = f64("f0_b") + f0.T @ bd
    colvecs = {
        "cp1": cp1, "cp2": cp2,
        "bv1": inputs["bv_m"], "bv2": inputs["bv_c"],
        "gm1": inputs["g_m"], "bm1": inputs["b_m"],
        "gm2": inputs["g_c"], "bm2": inputs["b_c"],
        "gd": inputs["g_d"], "bd": inputs["b_d"],
        "fb0": fb0, "fb1": inputs["f1_b"],
    }
    cols = np.empty((128, NCOL * 8 + 128), np.float32)
    for c, n in enumerate(COLS):
        cols[:, c * 8:(c + 1) * 8] = np.asarray(colvecs[n], np.float32) \
            .reshape(KD, 128).T
    cols[:, ONES128:] = 1.0
    colsb = np.zeros((128, 16), ml_dtypes.bfloat16)
    colsb[:, 0] = 1.0
    uxw = np.zeros((128, 2 * D), np.float32)
    uxw[0, 0:D] = -bf(wp2g).astype(np.float64).sum(0)
    uxw[0, D:2 * D] = -bf(f0g).astype(np.float64).sum(0)
    shared = {
        "w_p1": bf(wp1), "w_p2": bf(wp2g),
        "w_v1": bf(inputs["wv_m"]), "w_v2": bf(inputs["wv_c"]),
        "w_f0": bf(f0g), "w_f1": bf(inputs["f1_w"]),
        "cols": cols, "colsb": colsb, "uxw": bf(uxw),
    }
    in_maps = []
    for c in range(N_CORES):
        b, h = c // 2, c % 2
        xb = np.asarray(x[b], np.float32)
        yb = np.asarray(y[b], np.float32)
        xT = np.ascontiguousarray(xb.T)  # [D, T]
        yT = np.ascontiguousarray(yb.T)
        xqT = np.ascontiguousarray(xT[:, h * SH:(h + 1) * SH])
        m = dict(shared)
        m["xkv"] = bf(xT).reshape(KD, 128, T)
        m["ykv"] = bf(yT).reshape(KD, 128, T)
        m["xtok"] = bf(xb).reshape(TM, 128, D)
        m["ytok"] = bf(yb).reshape(TM, 128, D)
        m["xqf"] = xqT.reshape(KD, 128, SH)
        m["xqb"] = bf(xqT).reshape(KD, 128, SH)
        in_maps.append(m)
    return in_maps


def kernel(**inputs):
    nc = bacc.Bacc("TRN2", target_bir_lowering=False, debug=False,
                   num_devices=N_CORES)
    build_decoder(nc, taps=False)
    in_maps = _prep_inputs(inputs)
    res = run_bass_kernel_spmd(nc, in_maps, core_ids=list(range(N_CORES)),
                               trace=False)
    out = np.empty((B, S, D), np.float32)
    for c in range(N_CORES):
        b, h = c // 2, c % 2
        o = res.results[c]["out"].reshape(D, SH)  # feature-major [d, s]
        out[b, h * SH:(h + 1) * SH, :] = o.T
    return out
